# revision 16
# baseline (speedup 1.0000x reference)
"""MiniGPT (L=8, E=1024, H=16, T=1024, B=4, V=32000) on 8 TRN2 NeuronCores.

Sharding: data-parallel over (batch, sequence-half) -> 8 shards of 512 tokens.
All weights replicated per core. Per layer, the two cores sharing a batch
exchange K/V via pair AllGathers. Causal masking is data-driven (per-core
mask tables) so the SPMD program is uniform across cores.

v4: all-bf16 matmul operands, packed weight/mask/V tiles (one DMA per
block), 2-bank batched exp, depth-2 software-pipelined attention, LN
stats matmuls interleaved into the producer loops, PSUM rebalanced
(scores 2x2 banks / stats in the py pool / everything else on 2 banks).
"""
import sys

sys.path.insert(0, "/opt/trn_rl_repo")

import numpy as np
import ml_dtypes

import concourse.bass as bass
import concourse.bacc as bacc
import concourse.mybir as mybir
import concourse.tile as tile
from concourse.bass_utils import run_bass_kernel_spmd

V, E, H, L, T, B = 32000, 1024, 16, 8, 1024, 4
D = E // H              # 64
F = 4 * E               # 4096
EPS = 1e-5
TOK = 512               # tokens per core
NCORES = 8
ET = E // 128            # 8 feature tiles
FT = F // 128            # 32 mlp-hidden tiles
SCALE = 1.0 / np.sqrt(D)

F32 = mybir.dt.float32
F32R = mybir.dt.float32r
BF16 = mybir.dt.bfloat16
AF = mybir.ActivationFunctionType
ALU = mybir.AluOpType

PAIRS = [[0, 1], [2, 3], [4, 5], [6, 7]]
HEAD_CHUNKS = [(i * 512, 512) for i in range(62)] + [(62 * 512, 256)]

_CACHED = {}


def _build_nc():
    nc = bacc.Bacc("TRN2", target_bir_lowering=False, debug=False,
                   num_devices=NCORES)

    def P(name, shape, dt, out=False):
        return nc.declare_dram_parameter(name, list(shape), dt, isOutput=out)

    x0T = P("x0T", [E, TOK], F32R)                 # per-core residual seed
    wqkvT = P("wqkvT", [L, E, 3 * E], BF16)        # cols: [K | V | Q]
    wprojT = P("wprojT", [L, E, E], BF16)
    w1T = P("w1T", [L, E, F], BF16)
    w2T = P("w2T", [L, F, E], BF16)
    b1c = P("b1c", [L, 128, FT], F32)              # fc1 bias as columns
    b2c = P("b2c", [L, 128, ET], F32)              # fc2 bias as columns
    lnv = P("lnv", [L, 4, 128, ET], F32)           # ln1_g, ln1_b, ln2_g, ln2_b
    lnf = P("lnf", [2, 128, ET], F32)              # lnf_g, lnf_b
    headT = P("headT", [E, V], BF16)
    masks = P("masks", [8, 128, TOK], BF16)        # per-core causal masks
    ones_p = P("ones_p", [128, 16], F32R)          # all-ones helper
    logits = P("logits", [TOK, V], F32, out=True)

    with tile.TileContext(nc) as tc:
        with (
            tc.tile_pool(name="persist", bufs=1) as persist,
            tc.tile_pool(name="acts", bufs=8) as acts,         # h1/Y/h2 [128,512] bf16
            tc.tile_pool(name="qt", bufs=8) as qtp,            # QT [128,512] bf16
            tc.tile_pool(name="ut", bufs=FT) as utp,           # [128,512] bf16
            tc.tile_pool(name="wq", bufs=4) as wq,             # [128,8,512] bf16 packed
            tc.tile_pool(name="w2", bufs=2) as w2p,            # [128,32,128] bf16
            tc.tile_pool(name="stg", bufs=4) as stg,           # [128,512] f32 staging
            tc.tile_pool(name="pp", bufs=3) as pp,             # [128,8,512] bf16 probs
            tc.tile_pool(name="vec", bufs=2) as vec,
            tc.tile_pool(name="sm", bufs=6) as sm,             # [1,512] stats
            tc.tile_pool(name="ps", bufs=2, space="PSUM") as ps,
            tc.tile_pool(name="ps2", bufs=2, space="PSUM") as ps2,  # [128,1024]
            tc.tile_pool(name="psy", bufs=2, space="PSUM") as psy,
            tc.tile_pool(name="dram", bufs=8, space="DRAM") as dram,
        ):
            # ---- persistent tiles ----
            xT = [persist.tile([128, TOK], F32R, tag=f"xT{e}", name=f"xT{e}")
                  for e in range(ET)]
            # packed gathered K: [p, r(feat tile), rk(rank), t] bf16
            KTp = persist.tile([128, ET, 2, TOK], BF16, tag="KTp", name="KTp")
            # packed gathered V + ones column: [p, kt, h, d(65)]
            VSp = persist.tile([128, 8, H, 65], BF16, tag="VSp", name="VSp")
            MKp = persist.tile([128, 8, TOK], BF16, tag="MKp", name="MKp")
            xB = [persist.tile([128, TOK], BF16, tag=f"xB{e}", name=f"xB{e}")
                  for e in range(ET)]
            ones_col = persist.tile([128, 1], BF16, tag="ones_col")
            ones_row = persist.tile([1, 128], BF16, tag="ones_row")
            eps_t = persist.tile([1, 1], F32, tag="eps")
            nc.vector.memset(ones_col[:], 1.0)
            nc.vector.memset(ones_row[:], 1.0)
            nc.vector.memset(eps_t[:], EPS)
            nc.vector.memset(VSp[:, :, :, 64:65], 1.0)
            nc.sync.dma_start(out=MKp[:],
                              in_=masks.rearrange("k p t -> p k t"))

            for e in range(ET):
                nc.sync.dma_start(out=xT[e][:], in_=x0T[e * 128:(e + 1) * 128, :])
                with nc.allow_low_precision(reason="bf16 stats mirror"):
                    nc.vector.tensor_copy(xB[e][:], xT[e][:])

            # ---- layernorm split: per-tile stats accum + apply ----
            def ln_stats_new():
                psum = psy.tile([1, TOK], F32, tag="ybank")
                psq = psy.tile([1, TOK], F32, tag="ybank")
                return psum, psq

            def ln_stats_e(st, e, src_t):
                psum, psq = st
                sq = stg.tile([128, TOK], BF16, tag="stg")
                nc.scalar.activation(sq[:], src_t[:], AF.Square)
                nc.tensor.matmul(psum[:], lhsT=ones_col[:], rhs=src_t[:],
                                 start=(e == 0), stop=(e == ET - 1))
                nc.tensor.matmul(psq[:], lhsT=ones_col[:], rhs=sq[:],
                                 start=(e == 0), stop=(e == ET - 1))

            def ln_apply(st, src, g_ap, b_ap):
                psum, psq = st
                mu = sm.tile([1, TOK], F32, tag="sm")
                var = sm.tile([1, TOK], F32, tag="sm")
                mu2 = sm.tile([1, TOK], F32, tag="sm")
                rstd = sm.tile([1, TOK], F32, tag="sm")
                rstd_r = sm.tile([1, TOK], BF16, tag="sm")
                nmr = sm.tile([1, TOK], BF16, tag="sm")
                nc.scalar.activation(mu[:], psum[:], AF.Identity, scale=1.0 / E)
                nc.scalar.activation(var[:], psq[:], AF.Identity, scale=1.0 / E)
                nc.vector.tensor_mul(mu2[:], mu[:], mu[:])
                nc.vector.tensor_sub(var[:], var[:], mu2[:])
                nc.scalar.activation(var[:], var[:], AF.Sqrt, bias=eps_t[:])
                nc.vector.reciprocal_approx_fast(rstd[:], var[:])
                with nc.allow_low_precision(reason="f32r rounding"):
                    nc.vector.tensor_copy(rstd_r[:], rstd[:])
                    nc.vector.scalar_tensor_tensor(
                        out=nmr[:], in0=mu[:], scalar=-1.0, in1=rstd[:],
                        op0=ALU.mult, op1=ALU.mult)
                A = ps.tile([128, TOK], F32, tag="bank")
                C = ps.tile([128, TOK], F32, tag="bank")
                nc.tensor.matmul(A[:], lhsT=ones_row[:], rhs=rstd_r[:],
                                 start=True, stop=True)
                nc.tensor.matmul(C[:], lhsT=ones_row[:], rhs=nmr[:],
                                 start=True, stop=True)
                out = []
                for e in range(ET):
                    t1 = stg.tile([128, TOK], F32R, tag="stg")
                    h = acts.tile([128, TOK], BF16, tag="acts")
                    nc.vector.tensor_mul(t1[:], src[e][:], A[:])
                    nc.vector.tensor_add(t1[:], t1[:], C[:])
                    nc.scalar.activation(h[:], t1[:], AF.Identity,
                                         scale=g_ap[:, e:e + 1],
                                         bias=b_ap[:, e:e + 1])
                    out.append(h)
                return out

            def load_wblk(src2d, c0, vlen=512):
                """One DMA: [1024, vlen] block -> packed [128, 8, vlen] bf16."""
                wt = wq.tile([128, ET, 512], BF16, tag="wq", name="wt")
                nc.sync.dma_start(
                    out=wt[:, :, 0:vlen],
                    in_=src2d[:, c0:c0 + vlen].rearrange("(k p) c -> p k c",
                                                         p=128))
                return wt

            # seed LN1 stats for layer 0
            st1 = ln_stats_new()
            for e in range(ET):
                ln_stats_e(st1, e, xB[e])

            for l in range(L):
                lnt = vec.tile([128, 4 * ET], F32, tag="lnt")
                nc.sync.dma_start(out=lnt[:],
                                  in_=lnv[l].rearrange("a p b -> p a b"))
                b1t = vec.tile([128, FT], F32, tag="b1t")
                nc.sync.dma_start(out=b1t[:], in_=b1c[l])
                b2t = vec.tile([128, ET], F32, tag="b2t")
                nc.sync.dma_start(out=b2t[:], in_=b2c[l])

                # ---- LN1 ----
                h1 = ln_apply(st1, xT, lnt[:, 0:ET], lnt[:, ET:2 * ET])

                # ---- qkv: K rows, V, then Q ----
                stage_kh = [dram.tile([E // 2, TOK], BF16, tag=f"stgk{i}",
                                      name=f"stgk{i}")
                            for i in range(2)]
                full_kh = [dram.tile([2, E // 2, TOK], BF16, tag=f"fullk{i}",
                                     name=f"fullk{i}")
                           for i in range(2)]
                stage_v = dram.tile([TOK, E], BF16, tag="stgv")
                full_v = dram.tile([2, TOK, E], BF16, tag="fullv")

                for cb in range(2):          # K rows (wqkvT cols 0..1023)
                    wt = load_wblk(wqkvT[l], cb * 512)
                    for r in range(4):
                        pk = ps.tile([128, TOK], F32, tag="bank")
                        for k in range(ET):
                            nc.tensor.matmul(pk[:],
                                             lhsT=wt[:, k, r * 128:(r + 1) * 128],
                                             rhs=h1[k][:], start=(k == 0),
                                             stop=(k == ET - 1))
                        ksb = stg.tile([128, TOK], BF16, tag="stg")
                        with nc.allow_low_precision(reason="bf16 stage"):
                            nc.vector.tensor_copy(ksb[:], pk[:])
                        nc.sync.dma_start(
                            out=stage_kh[cb][r * 128:(r + 1) * 128, :],
                            in_=ksb[:])
                    nc.gpsimd.collective_compute(
                        "AllGather", ALU.bypass, replica_groups=PAIRS,
                        ins=[stage_kh[cb][:]], outs=[full_kh[cb][:]])

                for cb in range(2):          # V (wqkvT cols 1024..2047)
                    wt = load_wblk(wqkvT[l], 1024 + cb * 512)
                    for t in range(4):
                        pv = ps.tile([128, 512], F32, tag="bank")
                        for k in range(ET):
                            nc.tensor.matmul(pv[:],
                                             lhsT=h1[k][:, t * 128:(t + 1) * 128],
                                             rhs=wt[:, k, :], start=(k == 0),
                                             stop=(k == ET - 1))
                        vsb = stg.tile([128, 512], BF16, tag="stg")
                        with nc.allow_low_precision(reason="bf16 stage"):
                            nc.vector.tensor_copy(vsb[:], pv[:])
                        nc.sync.dma_start(
                            out=stage_v[t * 128:(t + 1) * 128,
                                        cb * 512:(cb + 1) * 512], in_=vsb[:])
                nc.gpsimd.collective_compute(
                    "AllGather", ALU.bypass, replica_groups=PAIRS,
                    ins=[stage_v[:]], outs=[full_v[:]])

                QT = []
                for cb in range(2):          # Q rows (wqkvT cols 2048..3071)
                    wt = load_wblk(wqkvT[l], 2048 + cb * 512)
                    for r in range(4):
                        pq = ps.tile([128, TOK], F32, tag="bank")
                        for k in range(ET):
                            nc.tensor.matmul(pq[:],
                                             lhsT=wt[:, k, r * 128:(r + 1) * 128],
                                             rhs=h1[k][:], start=(k == 0),
                                             stop=(k == ET - 1))
                        q = qtp.tile([128, TOK], BF16, tag="qt")
                        with nc.allow_low_precision(reason="bf16 q"):
                            nc.vector.tensor_copy(q[:], pq[:])
                        QT.append(q)

                # load gathered K/V (global key order: rank0 | rank1)
                for rk in range(2):
                    for i in range(2):
                        nc.sync.dma_start(
                            out=KTp[:, 4 * i:4 * (i + 1), rk, :],
                            in_=full_kh[i][rk].rearrange("(r p) t -> p r t",
                                                         p=128))
                    for t in range(4):
                        nc.sync.dma_start(
                            out=VSp[:, rk * 4 + t, :, 0:64],
                            in_=full_v[rk, t * 128:(t + 1) * 128, :].rearrange(
                                "p (h d) -> p h d", d=64))

                # ---- attention (depth-2 pipelined over heads) ----
                YT = [acts.tile([128, TOK], BF16, tag="acts", name="yt")
                      for _ in range(ET)]

                def attn_scores(h):
                    r, po = h // 2, (h % 2) * 64
                    prb = pp.tile([128, 8, TOK], BF16, tag="pp")
                    for j2 in range(4):
                        sc = ps2.tile([128, 2 * TOK], F32, tag="sc")
                        for u in range(2):
                            kt = 2 * j2 + u
                            rk, j = kt // 4, kt % 4
                            nc.tensor.matmul(
                                sc[:, u * TOK:(u + 1) * TOK],
                                lhsT=KTp[po:po + 64, r, rk,
                                         j * 128:(j + 1) * 128],
                                rhs=QT[r][po:po + 64, :],
                                start=True, stop=True)
                        nc.scalar.activation(prb[:, 2 * j2:2 * j2 + 2, :],
                                             sc[:], AF.Exp, scale=float(SCALE))
                    nc.vector.tensor_mul(prb[:], prb[:], MKp[:])
                    return prb

                def attn_pv(h, prb):
                    r, po = h // 2, (h % 2) * 64
                    py = psy.tile([65, TOK], F32, tag="ybank")
                    for kt in range(8):
                        nc.tensor.matmul(py[:], lhsT=VSp[:, kt, h, :],
                                         rhs=prb[:, kt, :],
                                         start=(kt == 0), stop=(kt == 7))
                    ysb = stg.tile([128, TOK], F32, tag="stg", name="ysb")
                    nc.vector.tensor_copy(ysb[0:64, :], py[0:64, :])
                    den0 = sm.tile([1, TOK], F32, tag="sm")
                    nc.vector.tensor_copy(den0[:], py[64:65, :])
                    rec = sm.tile([1, TOK], F32, tag="sm")
                    rec_r = sm.tile([1, TOK], BF16, tag="sm")
                    nc.vector.reciprocal_approx_fast(rec[:], den0[:])
                    with nc.allow_low_precision(reason="f32r rounding"):
                        nc.vector.tensor_copy(rec_r[:], rec[:])
                    pb = ps.tile([64, TOK], F32, tag="bank")
                    nc.tensor.matmul(pb[:], lhsT=ones_row[:, 0:64],
                                     rhs=rec_r[:], start=True, stop=True)
                    nc.vector.tensor_mul(YT[r][po:po + 64, :],
                                         ysb[0:64, :], pb[:])

                prb_a = attn_scores(0)
                prb_b = attn_scores(1)
                for h in range(2, H):
                    prb_c = attn_scores(h)
                    attn_pv(h - 2, prb_a)
                    prb_a, prb_b = prb_b, prb_c
                attn_pv(H - 2, prb_a)
                attn_pv(H - 1, prb_b)

                # ---- proj + residual (+ LN2 stats interleaved) ----
                st2 = ln_stats_new()
                for cb in range(2):
                    wt = load_wblk(wprojT[l], cb * 512)
                    for r in range(4):
                        e = cb * 4 + r
                        pe = ps.tile([128, TOK], F32, tag="bank")
                        for k in range(ET):
                            nc.tensor.matmul(pe[:],
                                             lhsT=wt[:, k, r * 128:(r + 1) * 128],
                                             rhs=YT[k][:], start=(k == 0),
                                             stop=(k == ET - 1))
                        nc.vector.tensor_add(xT[e][:], xT[e][:], pe[:])
                        with nc.allow_low_precision(reason="bf16 stats mirror"):
                            nc.vector.tensor_copy(xB[e][:], xT[e][:])
                        ln_stats_e(st2, e, xB[e])

                # ---- LN2 ----
                h2 = ln_apply(st2, xT, lnt[:, 2 * ET:3 * ET],
                              lnt[:, 3 * ET:4 * ET])

                # ---- fc1 + gelu ----
                uT = []
                for cb in range(8):
                    wt = load_wblk(w1T[l], cb * 512)
                    for r in range(4):
                        uc = cb * 4 + r
                        pu = ps.tile([128, TOK], F32, tag="bank")
                        for k in range(ET):
                            nc.tensor.matmul(pu[:],
                                             lhsT=wt[:, k, r * 128:(r + 1) * 128],
                                             rhs=h2[k][:], start=(k == 0),
                                             stop=(k == ET - 1))
                        u = utp.tile([128, TOK], BF16, tag="ut")
                        nc.scalar.activation(u[:], pu[:], AF.Gelu,
                                             bias=b1t[:, uc:uc + 1])
                        uT.append(u)

                # ---- fc2 + bias + residual (+ next LN1 stats) ----
                st1 = ln_stats_new()
                for e in range(ET):
                    w2t = w2p.tile([128, FT, 128], BF16, tag="w2", name="w2t")
                    nc.sync.dma_start(
                        out=w2t[:],
                        in_=w2T[l, :, e * 128:(e + 1) * 128].rearrange(
                            "(q p) e -> p q e", p=128))
                    pe = ps.tile([128, TOK], F32, tag="bank")
                    for uc in range(FT):
                        nc.tensor.matmul(
                            pe[:], lhsT=w2t[:, uc, :],
                            rhs=uT[uc][:], start=(uc == 0), stop=(uc == FT - 1))
                    nc.vector.scalar_tensor_tensor(
                        out=xT[e][:], in0=pe[:], scalar=b2t[:, e:e + 1],
                        in1=xT[e][:], op0=ALU.add, op1=ALU.add)
                    with nc.allow_low_precision(reason="bf16 stats mirror"):
                        nc.vector.tensor_copy(xB[e][:], xT[e][:])
                    ln_stats_e(st1, e, xB[e])

            # ---- final LN + head ----
            lnft = vec.tile([128, 2 * ET], F32, tag="lnft")
            nc.sync.dma_start(out=lnft[:], in_=lnf.rearrange("a p b -> p a b"))
            xf = ln_apply(st1, xT, lnft[:, 0:ET], lnft[:, ET:2 * ET])

            for (voff, vlen) in HEAD_CHUNKS:
                wt = wq.tile([128, ET, 512], BF16, tag="wq", name="wt")
                nc.sync.dma_start(
                    out=wt[:, :, 0:vlen],
                    in_=headT[:, voff:voff + vlen].rearrange(
                        "(k p) v -> p k v", p=128))
                for t in range(4):
                    pl = ps.tile([128, 512], F32, tag="bank")
                    for k in range(ET):
                        nc.tensor.matmul(pl[:, 0:vlen],
                                         lhsT=xf[k][:, t * 128:(t + 1) * 128],
                                         rhs=wt[:, k, 0:vlen], start=(k == 0),
                                         stop=(k == ET - 1))
                    lo = stg.tile([128, 512], F32, tag="stg")
                    nc.scalar.activation(lo[:, 0:vlen], pl[:, 0:vlen],
                                         AF.Identity)
                    nc.sync.dma_start(
                        out=logits[t * 128:(t + 1) * 128, voff:voff + vlen],
                        in_=lo[:, 0:vlen])

    nc.finalize()
    return nc


def _host_prep(inputs):
    """Build the 8 per-core input maps from the full model inputs."""
    idx = np.asarray(inputs["idx"])
    tok_emb = np.asarray(inputs["tok_emb"], np.float32)
    pos_emb = np.asarray(inputs["pos_emb"], np.float32)
    qkv_w = np.asarray(inputs["qkv_w"], np.float32)
    proj_w = np.asarray(inputs["proj_w"], np.float32)
    fc1_w = np.asarray(inputs["fc1_w"], np.float32)
    fc2_w = np.asarray(inputs["fc2_w"], np.float32)
    head_w = np.asarray(inputs["head_w"], np.float32)

    bf = ml_dtypes.bfloat16
    qkvT = np.ascontiguousarray(qkv_w.transpose(0, 2, 1))    # [L, E, 3E] (q,k,v)
    wqkvT = np.ascontiguousarray(
        np.concatenate([qkvT[:, :, E:2 * E], qkvT[:, :, 2 * E:3 * E],
                        qkvT[:, :, 0:E]], axis=2)).astype(bf)  # [K | V | Q]
    wprojT = np.ascontiguousarray(proj_w.transpose(0, 2, 1)).astype(bf)
    w1T = np.ascontiguousarray(fc1_w.transpose(0, 2, 1)).astype(bf)
    w2T = np.ascontiguousarray(fc2_w.transpose(0, 2, 1)).astype(bf)
    headTm = np.ascontiguousarray(head_w.T).astype(bf)        # [E, V]

    b1c = np.ascontiguousarray(
        np.asarray(inputs["fc1_b"], np.float32).reshape(L, FT, 128)
        .transpose(0, 2, 1))                                  # [L,128,FT]
    b2c = np.ascontiguousarray(
        np.asarray(inputs["fc2_b"], np.float32).reshape(L, ET, 128)
        .transpose(0, 2, 1))                                  # [L,128,ET]

    def cols(v):  # [L, E] -> [L, 128, ET]
        return np.ascontiguousarray(
            np.asarray(v, np.float32).reshape(L, ET, 128).transpose(0, 2, 1))

    lnv = np.ascontiguousarray(np.stack(
        [cols(inputs["ln1_g"]), cols(inputs["ln1_b"]),
         cols(inputs["ln2_g"]), cols(inputs["ln2_b"])], axis=1))
    lnf = np.ascontiguousarray(np.stack([
        np.asarray(inputs["lnf_g"], np.float32).reshape(ET, 128).T,
        np.asarray(inputs["lnf_b"], np.float32).reshape(ET, 128).T], axis=0))

    # causal mask tiles: M_j[p, f] = (p + 128*j <= f)
    p = np.arange(128)[:, None]
    f = np.arange(TOK)[None, :]
    mj = [(p + 128 * j <= f).astype(bf) for j in range(4)]
    zero = np.zeros((128, TOK), bf)
    one = np.ones((128, TOK), bf)
    m_half0 = np.stack(mj + [zero] * 4)      # visible: tiles 0..3 (diagonal)
    m_half1 = np.stack([one] * 4 + mj)       # tiles 0..3 past, 4..7 diagonal

    x0 = tok_emb[idx] + pos_emb[None, :, :]  # [B, T, E]

    shared = dict(wqkvT=wqkvT, wprojT=wprojT, w1T=w1T, w2T=w2T, b1c=b1c,
                  b2c=b2c, lnv=lnv, lnf=lnf, headT=headTm,
                  ones_p=np.ones((128, 16), np.float32))
    in_maps = []
    for c in range(NCORES):
        b, half = c // 2, c % 2
        m = dict(shared)
        m["x0T"] = np.ascontiguousarray(
            x0[b, half * TOK:(half + 1) * TOK, :].T).astype(np.float32)
        m["masks"] = np.ascontiguousarray(m_half0 if half == 0 else m_half1)
        in_maps.append(m)
    return in_maps


LAST_EXEC_NS = None


LAST_RES = None


def kernel(trace=False, trace_cores=None, tmpdir=None, **inputs) -> np.ndarray:
    global LAST_EXEC_NS, LAST_RES
    if "nc" not in _CACHED:
        _CACHED["nc"] = _build_nc()
    nc = _CACHED["nc"]
    in_maps = _host_prep(inputs)
    res = run_bass_kernel_spmd(nc, in_maps, core_ids=list(range(NCORES)),
                               trace=trace, trace_cores=trace_cores,
                               tmpdir=tmpdir)
    LAST_RES = res
    LAST_EXEC_NS = res.exec_time_ns
    out = np.empty((B, T, V), np.float32)
    for c in range(NCORES):
        b, half = c // 2, c % 2
        out[b, half * TOK:(half + 1) * TOK, :] = res.results[c]["logits"]
    return out


# revision 18
# speedup vs baseline: 1.0162x; 1.0162x over previous
"""MiniGPT (L=8, E=1024, H=16, T=1024, B=4, V=32000) on 8 TRN2 NeuronCores.

Sharding: data-parallel over (batch, sequence-half) -> 8 shards of 512 tokens.
All weights replicated per core. Per layer, the two cores sharing a batch
exchange K/V via pair AllGathers. Causal masking is data-driven (per-core
mask tables) so the SPMD program is uniform across cores.

v4: all-bf16 matmul operands, packed weight/mask/V tiles (one DMA per
block), 2-bank batched exp, depth-2 software-pipelined attention, LN
stats matmuls interleaved into the producer loops, PSUM rebalanced
(scores 2x2 banks / stats in the py pool / everything else on 2 banks).
"""
import sys

sys.path.insert(0, "/opt/trn_rl_repo")

import numpy as np
import ml_dtypes

import concourse.bass as bass
import concourse.bacc as bacc
import concourse.mybir as mybir
import concourse.tile as tile
from concourse.bass_utils import run_bass_kernel_spmd

V, E, H, L, T, B = 32000, 1024, 16, 8, 1024, 4
D = E // H              # 64
F = 4 * E               # 4096
EPS = 1e-5
TOK = 512               # tokens per core
NCORES = 8
ET = E // 128            # 8 feature tiles
FT = F // 128            # 32 mlp-hidden tiles
SCALE = 1.0 / np.sqrt(D)

F32 = mybir.dt.float32
F32R = mybir.dt.float32r
BF16 = mybir.dt.bfloat16
AF = mybir.ActivationFunctionType
ALU = mybir.AluOpType

PAIRS = [[0, 1], [2, 3], [4, 5], [6, 7]]
HEAD_CHUNKS = [(i * 512, 512) for i in range(62)] + [(62 * 512, 256)]

_CACHED = {}


def _build_nc():
    nc = bacc.Bacc("TRN2", target_bir_lowering=False, debug=False,
                   num_devices=NCORES)

    def P(name, shape, dt, out=False):
        return nc.declare_dram_parameter(name, list(shape), dt, isOutput=out)

    x0T = P("x0T", [E, TOK], F32R)                 # per-core residual seed
    wqkvT = P("wqkvT", [L, E, 3 * E], BF16)        # cols: [K | V | Q]
    wprojT = P("wprojT", [L, E, E], BF16)
    w1T = P("w1T", [L, E, F], BF16)
    w2T = P("w2T", [L, F, E], BF16)
    b1c = P("b1c", [L, 128, FT], F32)              # fc1 bias as columns
    b2c = P("b2c", [L, 128, ET], F32)              # fc2 bias as columns
    lnv = P("lnv", [L, 4, 128, ET], F32)           # ln1_g, ln1_b, ln2_g, ln2_b
    lnf = P("lnf", [2, 128, ET], F32)              # lnf_g, lnf_b
    headT = P("headT", [E, V], BF16)
    masks = P("masks", [8, 128, TOK], BF16)        # per-core causal masks
    ones_p = P("ones_p", [128, 16], F32R)          # all-ones helper
    logits = P("logits", [TOK, V], F32, out=True)

    with tile.TileContext(nc) as tc:
        with (
            tc.tile_pool(name="persist", bufs=1) as persist,
            tc.tile_pool(name="acts", bufs=8) as acts,         # h1/Y/h2 [128,512] bf16
            tc.tile_pool(name="qt", bufs=8) as qtp,            # QT [128,512] bf16
            tc.tile_pool(name="ut", bufs=FT) as utp,           # [128,512] bf16
            tc.tile_pool(name="wq", bufs=8) as wq,             # [128,8,512] bf16 packed
            tc.tile_pool(name="w2", bufs=2) as w2p,            # [128,32,128] bf16
            tc.tile_pool(name="stg", bufs=4) as stg,           # [128,512] f32 staging
            tc.tile_pool(name="pp", bufs=3) as pp,             # [128,8,512] bf16 probs
            tc.tile_pool(name="vec", bufs=2) as vec,
            tc.tile_pool(name="sm", bufs=6) as sm,             # [1,512] stats
            tc.tile_pool(name="ps", bufs=2, space="PSUM") as ps,
            tc.tile_pool(name="ps2", bufs=2, space="PSUM") as ps2,  # [128,1024]
            tc.tile_pool(name="psy", bufs=2, space="PSUM") as psy,
            tc.tile_pool(name="dram", bufs=8, space="DRAM") as dram,
        ):
            # ---- persistent tiles ----
            xT = [persist.tile([128, TOK], F32R, tag=f"xT{e}", name=f"xT{e}")
                  for e in range(ET)]
            # packed gathered K: [p, r(feat tile), rk(rank), t] bf16
            KTp = persist.tile([128, ET, 2, TOK], BF16, tag="KTp", name="KTp")
            # packed gathered V + ones column: [p, kt, h, d(65)]
            VSp = persist.tile([128, 8, H, 65], BF16, tag="VSp", name="VSp")
            MKp = persist.tile([128, 8, TOK], BF16, tag="MKp", name="MKp")
            xB = [persist.tile([128, TOK], BF16, tag=f"xB{e}", name=f"xB{e}")
                  for e in range(ET)]
            ones_col = persist.tile([128, 1], BF16, tag="ones_col")
            ones_row = persist.tile([1, 128], BF16, tag="ones_row")
            eps_t = persist.tile([1, 1], F32, tag="eps")
            nc.vector.memset(ones_col[:], 1.0)
            nc.vector.memset(ones_row[:], 1.0)
            nc.vector.memset(eps_t[:], EPS)
            nc.vector.memset(VSp[:, :, :, 64:65], 1.0)
            nc.sync.dma_start(out=MKp[:],
                              in_=masks.rearrange("k p t -> p k t"))

            for e in range(ET):
                nc.sync.dma_start(out=xT[e][:], in_=x0T[e * 128:(e + 1) * 128, :])
                with nc.allow_low_precision(reason="bf16 stats mirror"):
                    nc.vector.tensor_copy(xB[e][:], xT[e][:])

            # ---- layernorm split: per-tile stats accum + apply ----
            def ln_stats_new():
                psum = psy.tile([1, TOK], F32, tag="ybank")
                psq = psy.tile([1, TOK], F32, tag="ybank")
                return psum, psq

            def ln_stats_e(st, e, src_t):
                psum, psq = st
                sq = stg.tile([128, TOK], BF16, tag="stg")
                nc.scalar.activation(sq[:], src_t[:], AF.Square)
                nc.tensor.matmul(psum[:], lhsT=ones_col[:], rhs=src_t[:],
                                 start=(e == 0), stop=(e == ET - 1))
                nc.tensor.matmul(psq[:], lhsT=ones_col[:], rhs=sq[:],
                                 start=(e == 0), stop=(e == ET - 1))

            def ln_apply(st, src, g_ap, b_ap):
                psum, psq = st
                mu = sm.tile([1, TOK], F32, tag="sm")
                var = sm.tile([1, TOK], F32, tag="sm")
                mu2 = sm.tile([1, TOK], F32, tag="sm")
                rstd = sm.tile([1, TOK], F32, tag="sm")
                rstd_r = sm.tile([1, TOK], BF16, tag="sm")
                nmr = sm.tile([1, TOK], BF16, tag="sm")
                nc.scalar.activation(mu[:], psum[:], AF.Identity, scale=1.0 / E)
                nc.scalar.activation(var[:], psq[:], AF.Identity, scale=1.0 / E)
                nc.vector.tensor_mul(mu2[:], mu[:], mu[:])
                nc.vector.tensor_sub(var[:], var[:], mu2[:])
                nc.scalar.activation(var[:], var[:], AF.Sqrt, bias=eps_t[:])
                nc.vector.reciprocal_approx_fast(rstd[:], var[:])
                with nc.allow_low_precision(reason="f32r rounding"):
                    nc.vector.tensor_copy(rstd_r[:], rstd[:])
                    nc.vector.scalar_tensor_tensor(
                        out=nmr[:], in0=mu[:], scalar=-1.0, in1=rstd[:],
                        op0=ALU.mult, op1=ALU.mult)
                A = ps.tile([128, TOK], F32, tag="bank")
                C = ps.tile([128, TOK], F32, tag="bank")
                nc.tensor.matmul(A[:], lhsT=ones_row[:], rhs=rstd_r[:],
                                 start=True, stop=True)
                nc.tensor.matmul(C[:], lhsT=ones_row[:], rhs=nmr[:],
                                 start=True, stop=True)
                out = []
                for e in range(ET):
                    t1 = stg.tile([128, TOK], F32R, tag="stg")
                    h = acts.tile([128, TOK], BF16, tag="acts")
                    nc.vector.tensor_mul(t1[:], src[e][:], A[:])
                    nc.vector.tensor_add(t1[:], t1[:], C[:])
                    nc.scalar.activation(h[:], t1[:], AF.Identity,
                                         scale=g_ap[:, e:e + 1],
                                         bias=b_ap[:, e:e + 1])
                    out.append(h)
                return out

            def load_wblk(src2d, c0, vlen=512):
                """Two DMAs: [1024, vlen] block -> 2x packed [128, 4, vlen]
                half-block tiles (finer prefetch granularity)."""
                halves = []
                for kh in range(2):
                    wt = wq.tile([128, 4, 512], BF16, tag="wq", name="wt")
                    nc.sync.dma_start(
                        out=wt[:, :, 0:vlen],
                        in_=src2d[kh * 512:(kh + 1) * 512, c0:c0 + vlen]
                        .rearrange("(k p) c -> p k c", p=128))
                    halves.append(wt)

                class WView:
                    def __getitem__(self, idx):
                        p, k, c = idx
                        return halves[k // 4][p, k % 4, c]
                return WView()

            # seed LN1 stats for layer 0
            st1 = ln_stats_new()
            for e in range(ET):
                ln_stats_e(st1, e, xB[e])

            for l in range(L):
                lnt = vec.tile([128, 4 * ET], F32, tag="lnt")
                nc.sync.dma_start(out=lnt[:],
                                  in_=lnv[l].rearrange("a p b -> p a b"))
                b1t = vec.tile([128, FT], F32, tag="b1t")
                nc.sync.dma_start(out=b1t[:], in_=b1c[l])
                b2t = vec.tile([128, ET], F32, tag="b2t")
                nc.sync.dma_start(out=b2t[:], in_=b2c[l])

                # ---- LN1 ----
                h1 = ln_apply(st1, xT, lnt[:, 0:ET], lnt[:, ET:2 * ET])

                # ---- qkv: K rows, V, then Q ----
                stage_kh = [dram.tile([E // 2, TOK], BF16, tag=f"stgk{i}",
                                      name=f"stgk{i}")
                            for i in range(2)]
                full_kh = [dram.tile([2, E // 2, TOK], BF16, tag=f"fullk{i}",
                                     name=f"fullk{i}")
                           for i in range(2)]
                stage_v = dram.tile([TOK, E], BF16, tag="stgv")
                full_v = dram.tile([2, TOK, E], BF16, tag="fullv")

                for cb in range(2):          # K rows (wqkvT cols 0..1023)
                    wt = load_wblk(wqkvT[l], cb * 512)
                    for r in range(4):
                        pk = ps.tile([128, TOK], F32, tag="bank")
                        for k in range(ET):
                            nc.tensor.matmul(pk[:],
                                             lhsT=wt[:, k, r * 128:(r + 1) * 128],
                                             rhs=h1[k][:], start=(k == 0),
                                             stop=(k == ET - 1))
                        ksb = stg.tile([128, TOK], BF16, tag="stg")
                        with nc.allow_low_precision(reason="bf16 stage"):
                            nc.vector.tensor_copy(ksb[:], pk[:])
                        nc.sync.dma_start(
                            out=stage_kh[cb][r * 128:(r + 1) * 128, :],
                            in_=ksb[:])
                    nc.gpsimd.collective_compute(
                        "AllGather", ALU.bypass, replica_groups=PAIRS,
                        ins=[stage_kh[cb][:]], outs=[full_kh[cb][:]])

                for cb in range(2):          # V (wqkvT cols 1024..2047)
                    wt = load_wblk(wqkvT[l], 1024 + cb * 512)
                    for t in range(4):
                        pv = ps.tile([128, 512], F32, tag="bank")
                        for k in range(ET):
                            nc.tensor.matmul(pv[:],
                                             lhsT=h1[k][:, t * 128:(t + 1) * 128],
                                             rhs=wt[:, k, :], start=(k == 0),
                                             stop=(k == ET - 1))
                        vsb = stg.tile([128, 512], BF16, tag="stg")
                        with nc.allow_low_precision(reason="bf16 stage"):
                            nc.vector.tensor_copy(vsb[:], pv[:])
                        nc.sync.dma_start(
                            out=stage_v[t * 128:(t + 1) * 128,
                                        cb * 512:(cb + 1) * 512], in_=vsb[:])
                nc.gpsimd.collective_compute(
                    "AllGather", ALU.bypass, replica_groups=PAIRS,
                    ins=[stage_v[:]], outs=[full_v[:]])

                QT = []
                for cb in range(2):          # Q rows (wqkvT cols 2048..3071)
                    wt = load_wblk(wqkvT[l], 2048 + cb * 512)
                    for r in range(4):
                        pq = ps.tile([128, TOK], F32, tag="bank")
                        for k in range(ET):
                            nc.tensor.matmul(pq[:],
                                             lhsT=wt[:, k, r * 128:(r + 1) * 128],
                                             rhs=h1[k][:], start=(k == 0),
                                             stop=(k == ET - 1))
                        q = qtp.tile([128, TOK], BF16, tag="qt")
                        with nc.allow_low_precision(reason="bf16 q"):
                            nc.vector.tensor_copy(q[:], pq[:])
                        QT.append(q)

                # load gathered K/V (global key order: rank0 | rank1)
                for rk in range(2):
                    for i in range(2):
                        nc.sync.dma_start(
                            out=KTp[:, 4 * i:4 * (i + 1), rk, :],
                            in_=full_kh[i][rk].rearrange("(r p) t -> p r t",
                                                         p=128))
                    for t in range(4):
                        nc.sync.dma_start(
                            out=VSp[:, rk * 4 + t, :, 0:64],
                            in_=full_v[rk, t * 128:(t + 1) * 128, :].rearrange(
                                "p (h d) -> p h d", d=64))

                # ---- attention (depth-2 pipelined over heads) ----
                YT = [acts.tile([128, TOK], BF16, tag="acts", name="yt")
                      for _ in range(ET)]

                def attn_scores(h):
                    r, po = h // 2, (h % 2) * 64
                    prb = pp.tile([128, 8, TOK], BF16, tag="pp")
                    for j2 in range(4):
                        sc = ps2.tile([128, 2 * TOK], F32, tag="sc")
                        for u in range(2):
                            kt = 2 * j2 + u
                            rk, j = kt // 4, kt % 4
                            nc.tensor.matmul(
                                sc[:, u * TOK:(u + 1) * TOK],
                                lhsT=KTp[po:po + 64, r, rk,
                                         j * 128:(j + 1) * 128],
                                rhs=QT[r][po:po + 64, :],
                                start=True, stop=True)
                        nc.scalar.activation(prb[:, 2 * j2:2 * j2 + 2, :],
                                             sc[:], AF.Exp, scale=float(SCALE))
                    nc.vector.tensor_mul(prb[:], prb[:], MKp[:])
                    return prb

                def attn_pv(h, prb):
                    r, po = h // 2, (h % 2) * 64
                    py = psy.tile([65, TOK], F32, tag="ybank")
                    for kt in range(8):
                        nc.tensor.matmul(py[:], lhsT=VSp[:, kt, h, :],
                                         rhs=prb[:, kt, :],
                                         start=(kt == 0), stop=(kt == 7))
                    ysb = stg.tile([128, TOK], F32, tag="stg", name="ysb")
                    nc.vector.tensor_copy(ysb[0:64, :], py[0:64, :])
                    den0 = sm.tile([1, TOK], F32, tag="sm")
                    nc.vector.tensor_copy(den0[:], py[64:65, :])
                    rec = sm.tile([1, TOK], F32, tag="sm")
                    rec_r = sm.tile([1, TOK], BF16, tag="sm")
                    nc.vector.reciprocal_approx_fast(rec[:], den0[:])
                    with nc.allow_low_precision(reason="f32r rounding"):
                        nc.vector.tensor_copy(rec_r[:], rec[:])
                    pb = ps.tile([64, TOK], F32, tag="bank")
                    nc.tensor.matmul(pb[:], lhsT=ones_row[:, 0:64],
                                     rhs=rec_r[:], start=True, stop=True)
                    nc.vector.tensor_mul(YT[r][po:po + 64, :],
                                         ysb[0:64, :], pb[:])

                prb_a = attn_scores(0)
                prb_b = attn_scores(1)
                for h in range(2, H):
                    prb_c = attn_scores(h)
                    attn_pv(h - 2, prb_a)
                    prb_a, prb_b = prb_b, prb_c
                attn_pv(H - 2, prb_a)
                attn_pv(H - 1, prb_b)

                # ---- proj + residual (+ LN2 stats interleaved) ----
                st2 = ln_stats_new()
                for cb in range(2):
                    wt = load_wblk(wprojT[l], cb * 512)
                    for r in range(4):
                        e = cb * 4 + r
                        pe = ps.tile([128, TOK], F32, tag="bank")
                        for k in range(ET):
                            nc.tensor.matmul(pe[:],
                                             lhsT=wt[:, k, r * 128:(r + 1) * 128],
                                             rhs=YT[k][:], start=(k == 0),
                                             stop=(k == ET - 1))
                        nc.vector.tensor_add(xT[e][:], xT[e][:], pe[:])
                        with nc.allow_low_precision(reason="bf16 stats mirror"):
                            nc.vector.tensor_copy(xB[e][:], xT[e][:])
                        ln_stats_e(st2, e, xB[e])

                # ---- LN2 ----
                h2 = ln_apply(st2, xT, lnt[:, 2 * ET:3 * ET],
                              lnt[:, 3 * ET:4 * ET])

                # ---- fc1 + gelu ----
                uT = []
                for cb in range(8):
                    wt = load_wblk(w1T[l], cb * 512)
                    for r in range(4):
                        uc = cb * 4 + r
                        pu = ps.tile([128, TOK], F32, tag="bank")
                        for k in range(ET):
                            nc.tensor.matmul(pu[:],
                                             lhsT=wt[:, k, r * 128:(r + 1) * 128],
                                             rhs=h2[k][:], start=(k == 0),
                                             stop=(k == ET - 1))
                        u = utp.tile([128, TOK], BF16, tag="ut")
                        nc.scalar.activation(u[:], pu[:], AF.Gelu,
                                             bias=b1t[:, uc:uc + 1])
                        uT.append(u)

                # ---- fc2 + bias + residual (+ next LN1 stats) ----
                st1 = ln_stats_new()
                for e in range(ET):
                    w2t = w2p.tile([128, FT, 128], BF16, tag="w2", name="w2t")
                    nc.sync.dma_start(
                        out=w2t[:],
                        in_=w2T[l, :, e * 128:(e + 1) * 128].rearrange(
                            "(q p) e -> p q e", p=128))
                    pe = ps.tile([128, TOK], F32, tag="bank")
                    for uc in range(FT):
                        nc.tensor.matmul(
                            pe[:], lhsT=w2t[:, uc, :],
                            rhs=uT[uc][:], start=(uc == 0), stop=(uc == FT - 1))
                    nc.vector.scalar_tensor_tensor(
                        out=xT[e][:], in0=pe[:], scalar=b2t[:, e:e + 1],
                        in1=xT[e][:], op0=ALU.add, op1=ALU.add)
                    with nc.allow_low_precision(reason="bf16 stats mirror"):
                        nc.vector.tensor_copy(xB[e][:], xT[e][:])
                    ln_stats_e(st1, e, xB[e])

            # ---- final LN + head ----
            lnft = vec.tile([128, 2 * ET], F32, tag="lnft")
            nc.sync.dma_start(out=lnft[:], in_=lnf.rearrange("a p b -> p a b"))
            xf = ln_apply(st1, xT, lnft[:, 0:ET], lnft[:, ET:2 * ET])

            for (voff, vlen) in HEAD_CHUNKS:
                wt = load_wblk(headT, voff, vlen)
                for t in range(4):
                    pl = ps.tile([128, 512], F32, tag="bank")
                    for k in range(ET):
                        nc.tensor.matmul(pl[:, 0:vlen],
                                         lhsT=xf[k][:, t * 128:(t + 1) * 128],
                                         rhs=wt[:, k, 0:vlen], start=(k == 0),
                                         stop=(k == ET - 1))
                    lo = stg.tile([128, 512], F32, tag="stg")
                    nc.scalar.activation(lo[:, 0:vlen], pl[:, 0:vlen],
                                         AF.Identity)
                    nc.sync.dma_start(
                        out=logits[t * 128:(t + 1) * 128, voff:voff + vlen],
                        in_=lo[:, 0:vlen])

    nc.finalize()
    return nc


def _host_prep(inputs):
    """Build the 8 per-core input maps from the full model inputs."""
    idx = np.asarray(inputs["idx"])
    tok_emb = np.asarray(inputs["tok_emb"], np.float32)
    pos_emb = np.asarray(inputs["pos_emb"], np.float32)
    qkv_w = np.asarray(inputs["qkv_w"], np.float32)
    proj_w = np.asarray(inputs["proj_w"], np.float32)
    fc1_w = np.asarray(inputs["fc1_w"], np.float32)
    fc2_w = np.asarray(inputs["fc2_w"], np.float32)
    head_w = np.asarray(inputs["head_w"], np.float32)

    bf = ml_dtypes.bfloat16
    qkvT = np.ascontiguousarray(qkv_w.transpose(0, 2, 1))    # [L, E, 3E] (q,k,v)
    wqkvT = np.ascontiguousarray(
        np.concatenate([qkvT[:, :, E:2 * E], qkvT[:, :, 2 * E:3 * E],
                        qkvT[:, :, 0:E]], axis=2)).astype(bf)  # [K | V | Q]
    wprojT = np.ascontiguousarray(proj_w.transpose(0, 2, 1)).astype(bf)
    w1T = np.ascontiguousarray(fc1_w.transpose(0, 2, 1)).astype(bf)
    w2T = np.ascontiguousarray(fc2_w.transpose(0, 2, 1)).astype(bf)
    headTm = np.ascontiguousarray(head_w.T).astype(bf)        # [E, V]

    b1c = np.ascontiguousarray(
        np.asarray(inputs["fc1_b"], np.float32).reshape(L, FT, 128)
        .transpose(0, 2, 1))                                  # [L,128,FT]
    b2c = np.ascontiguousarray(
        np.asarray(inputs["fc2_b"], np.float32).reshape(L, ET, 128)
        .transpose(0, 2, 1))                                  # [L,128,ET]

    def cols(v):  # [L, E] -> [L, 128, ET]
        return np.ascontiguousarray(
            np.asarray(v, np.float32).reshape(L, ET, 128).transpose(0, 2, 1))

    lnv = np.ascontiguousarray(np.stack(
        [cols(inputs["ln1_g"]), cols(inputs["ln1_b"]),
         cols(inputs["ln2_g"]), cols(inputs["ln2_b"])], axis=1))
    lnf = np.ascontiguousarray(np.stack([
        np.asarray(inputs["lnf_g"], np.float32).reshape(ET, 128).T,
        np.asarray(inputs["lnf_b"], np.float32).reshape(ET, 128).T], axis=0))

    # causal mask tiles: M_j[p, f] = (p + 128*j <= f)
    p = np.arange(128)[:, None]
    f = np.arange(TOK)[None, :]
    mj = [(p + 128 * j <= f).astype(bf) for j in range(4)]
    zero = np.zeros((128, TOK), bf)
    one = np.ones((128, TOK), bf)
    m_half0 = np.stack(mj + [zero] * 4)      # visible: tiles 0..3 (diagonal)
    m_half1 = np.stack([one] * 4 + mj)       # tiles 0..3 past, 4..7 diagonal

    x0 = tok_emb[idx] + pos_emb[None, :, :]  # [B, T, E]

    shared = dict(wqkvT=wqkvT, wprojT=wprojT, w1T=w1T, w2T=w2T, b1c=b1c,
                  b2c=b2c, lnv=lnv, lnf=lnf, headT=headTm,
                  ones_p=np.ones((128, 16), np.float32))
    in_maps = []
    for c in range(NCORES):
        b, half = c // 2, c % 2
        m = dict(shared)
        m["x0T"] = np.ascontiguousarray(
            x0[b, half * TOK:(half + 1) * TOK, :].T).astype(np.float32)
        m["masks"] = np.ascontiguousarray(m_half0 if half == 0 else m_half1)
        in_maps.append(m)
    return in_maps


LAST_EXEC_NS = None


LAST_RES = None


def kernel(trace=False, trace_cores=None, tmpdir=None, **inputs) -> np.ndarray:
    global LAST_EXEC_NS, LAST_RES
    if "nc" not in _CACHED:
        _CACHED["nc"] = _build_nc()
    nc = _CACHED["nc"]
    in_maps = _host_prep(inputs)
    res = run_bass_kernel_spmd(nc, in_maps, core_ids=list(range(NCORES)),
                               trace=trace, trace_cores=trace_cores,
                               tmpdir=tmpdir)
    LAST_RES = res
    LAST_EXEC_NS = res.exec_time_ns
    out = np.empty((B, T, V), np.float32)
    for c in range(NCORES):
        b, half = c // 2, c % 2
        out[b, half * TOK:(half + 1) * TOK, :] = res.results[c]["logits"]
    return out
